# revision 27
# baseline (speedup 1.0000x reference)
"""Trainium2 Bass kernel for nn_CrossAttention (16-head cross attention).

Reference computation (fp32, s1=s2=2048, d1=d2=1024, H=16, DK=DV=64):
    q = x1 @ Wq.T ; k = x2 @ Wk.T ; v = x2 @ Wv.T      (per-head reshape)
    attn = softmax(q k^T / 8) per head
    out = LeakyReLU_0.01((attn v) @ Wo.T + bo)

Distribution (8 NeuronCores, tensor-parallel over heads):
  - Each core owns 2 heads: column-shards of Wq/Wk/Wv (128 rows each).
  - Inputs are fed pre-transposed from the host (x1.T, x2.T, W.T) so the
    contraction axis lands on SBUF partitions without any on-device
    transposition of the big activations.
  - Per-head attention computed locally in "transposed" orientation:
    S^T[j,i] tiles -> exp on ACT (no max subtraction needed: |scores|<~3
    by construction) -> O'^T = [V|1]^T @ P^T which fuses the softmax
    denominator into the matmul (row 64 of the PSUM output = row sums).
  - Normalized heads are exchanged with per-head AllToAlls in bf16
    (0.5 MB/core each) so each core ends up with ALL heads for its
    256-row slice of s1; the output projection then uses the full Wo
    (no reduction collective needed).
  - Wo is reordered on the host (even heads first, then odd) so the
    out-projection's first 4 K-blocks depend only on the head-0 AllToAll:
    that half runs while the head-1 AllToAll is still in flight.
  - Epilogue (bias via K=1 ones-row matmul + leaky relu), output is the
    core's 256-row slice; the host concatenates the 8 slices.

Engine/queue assignment: x1 loads on sync/HWDGE, x2 loads on gpsimd/SWDGE
(runs them in parallel; projections interleave q/k/v so both streams are
hot), weights on scalar/HWDGE, exchange-path DMAs on sync. The softmax
finalize uses reciprocal_approx_fast (~5x faster than DVE reciprocal) and
a K=1 f32r matmul to broadcast 1/Z across partitions, so phase-boundary
tensor stalls are ~1us instead of ~6us.
"""

import numpy as np

import concourse.bass as bass
import concourse.mybir as mybir
import concourse.tile as tile
from concourse import bacc
from concourse import bass_utils
from concourse.masks import make_identity

NC_CORES = 8
S1 = 2048
S2 = 2048
D1 = 1024
D2 = 1024
H, DK, DV = 16, 64, 64
HPC = H // NC_CORES          # heads per core = 2
EPC = HPC * DK               # projection dims per core = 128
SPC = S1 // NC_CORES         # output rows per core = 256
P = 128
F32 = mybir.dt.float32
F32R = mybir.dt.float32r
BF16 = mybir.dt.bfloat16
ACT_EXP = mybir.ActivationFunctionType.Exp
ACT_LN = mybir.ActivationFunctionType.Ln
ACT_LRELU = mybir.ActivationFunctionType.Lrelu
MAX = mybir.AluOpType.max

NEG_SLOPE = 0.01
SCALE = 1.0 / np.sqrt(np.float32(DK))   # 0.125

S2_T = S2 // P               # 16 key tiles
KD1 = D1 // P                # 8 contraction tiles for projections
KDV = (H * DV) // P          # 8 contraction tiles for out projection


def build(single_core: bool = False):
    """single_core=True swaps the AllToAll for a local DMA copy (its exact
    1-core semantics) so the kernel can run in TimelineSim for perf
    estimation."""
    import os as _os
    nc = bacc.Bacc("TRN2", target_bir_lowering=False, debug=False,
                   num_devices=1 if single_core else NC_CORES)

    # weights arrive host-pre-shuffled into the exact SBUF layout
    # (partition-major) so their DMAs are 128 big descriptors, not
    # thousands of 256B ones
    x1T = nc.dram_tensor("x1T", [D1, S1], BF16, kind="ExternalInput")
    x2T = nc.dram_tensor("x2T", [D2, S2], BF16, kind="ExternalInput")
    wqT = nc.dram_tensor("wqT", [P, KD1, EPC], BF16, kind="ExternalInput")
    wkT = nc.dram_tensor("wkT", [P, KD1, EPC], BF16, kind="ExternalInput")
    wvT = nc.dram_tensor("wvT", [P, KD1, EPC], BF16, kind="ExternalInput")
    woT = nc.dram_tensor("woT", [P, KDV, D1], BF16, kind="ExternalInput")
    bo_bc = nc.dram_tensor("bo_bc", [1, D1], F32, kind="ExternalInput")
    out = nc.dram_tensor("out", [SPC, D1], F32, kind="ExternalOutput")
    a2a_in = [nc.dram_tensor(f"a2a_in{h}", [NC_CORES * DV, SPC], BF16,
                             kind="Internal") for h in range(HPC)]
    a2a_out = [nc.dram_tensor(f"a2a_out{h}", [NC_CORES * DV, SPC], BF16,
                              kind="Internal") for h in range(HPC)]
    # tiny scratch collective fired at kernel start to absorb the CC
    # rendezvous + first-trigger setup (~12us) off the critical path
    warm_in = nc.dram_tensor("warm_in", [NC_CORES, 16], BF16, kind="Internal")
    warm_out = nc.dram_tensor("warm_out", [NC_CORES, 16], BF16,
                              kind="Internal")

    _ptb = int(_os.environ.get("PTB", "5"))
    _xtb = int(_os.environ.get("XTB", "3"))
    _psb = int(_os.environ.get("PSB", "3"))
    _pob = int(_os.environ.get("POB", "2"))
    _dgr = int(_os.environ.get("DGR", "2"))

    with tile.TileContext(nc) as tc:
        with (
            tc.tile_pool(name="const", bufs=1) as cpool,
            tc.tile_pool(name="res", bufs=1) as rpool,
            tc.tile_pool(name="x1p", bufs=_xtb) as x1pool,
            tc.tile_pool(name="x2p", bufs=_xtb) as x2pool,
            tc.tile_pool(name="lhs", bufs=1) as lpool,
            tc.tile_pool(name="pt", bufs=_ptb) as ptpool,
            tc.tile_pool(name="ytmp", bufs=1) as ypool,
            tc.tile_pool(name="norm", bufs=4) as npool,
            tc.tile_pool(name="big", bufs=1) as bpool,
            tc.tile_pool(name="ps", bufs=_psb, space="PSUM") as pspool,
            tc.tile_pool(name="po", bufs=_pob, space="PSUM") as popool,
        ):
            # ---------------- constants ----------------
            ident = cpool.tile([P, P], BF16)
            make_identity(nc, ident[:])
            wq_sb = cpool.tile([P, KD1, EPC], BF16)
            wk_sb = cpool.tile([P, KD1, EPC], BF16)
            wv_sb = cpool.tile([P, KD1, EPC], BF16)
            # weights on the scalar/HWDGE queue: the x1 (sync) and x2
            # (gpsimd swdge q0/q1) streams start immediately in parallel
            nc.scalar.dma_start(wk_sb[:], wkT[:])
            nc.scalar.dma_start(wv_sb[:], wvT[:])
            nc.scalar.dma_start(wq_sb[:], wqT[:])
            # wo is loaded later on the sync queue (after the x1 stream) so
            # it doesn't delay the first x2 tiles here on the scalar queue
            wo_sb = cpool.tile([P, KDV, D1], BF16)
            # bias as a single f32r row: added into the out-proj PSUM via a
            # K=1 ones-row matmul, so the epilogue is one max() on DVE
            bo_r = cpool.tile([1, D1], F32R)
            nc.gpsimd.dma_start(bo_r[:], bo_bc[0:1, :])
            if not single_core:
                nc.gpsimd.collective_compute(
                    "AllToAll", mybir.AluOpType.bypass,
                    replica_groups=[list(range(NC_CORES))],
                    ins=[warm_in[:].opt()],
                    outs=[warm_out[:].opt()],
                )
            ones_r = cpool.tile([1, P], F32R)
            nc.vector.memset(ones_r[:].bitcast(F32), 1.0)
            # all-ones row at partition DV used as K=1 matmul lhsT to
            # broadcast the softmax-denominator reciprocal across partitions
            ones_t = cpool.tile([DV + 1, DV], F32R)
            nc.vector.memset(ones_t[:].bitcast(F32), 1.0)

            # ---------------- residents ----------------
            # kTb/qTb live through all attention phases; vT dies after the
            # v transposes so its buffer is recycled for the two bf16
            # attention outputs (disjoint live ranges).
            kTb = bpool.tile([P, S2], BF16, tag="ktb", name="kTb")
            qTb = bpool.tile([P, S1], BF16, tag="qtb", name="qTb")
            vT = bpool.tile([P, S2], BF16, tag="big", name="vT")
            # V natural + ones column, per key tile: [j, (v_h0|1|v_h1|1)]
            vP = rpool.tile([P, S2_T, 2 * (DV + 1)], BF16)
            oTh = [bpool.tile([DV, S1], BF16, tag="big", name=f"oT{h}")
                   for h in range(HPC)]

            # ---------------- projections ----------------
            # K and V share one pass over x2T (each x2 tile DMA'd once).
            # x2 loads issue from gpsimd/SWDGE, x1 loads from sync/HWDGE;
            # kv and q projections interleave so both streams run in
            # parallel.
            x2v = x2T.rearrange("(o p) i -> p o i", p=P)
            x1v = x1T.rearrange("(o p) i -> p o i", p=P)

            kv_ps = {}
            q_ps = {}

            def kv_dg(gp, dg):
                gsl = slice(gp * 1024, (gp + 1) * 1024)
                if dg == 0:
                    kv_ps[gp] = (
                        pspool.tile([P, 1024], F32, tag="ps", name=f"pk{gp}"),
                        pspool.tile([P, 1024], F32, tag="ps", name=f"pv{gp}"),
                    )
                pk, pv = kv_ps[gp]
                xt = x2pool.tile([P, _dgr, 1024], BF16, tag="xt",
                                 name=f"xt2_{gp}_{dg}")
                # group-0 tiles split gpsimd-SWDGE (first two)/scalar-HWDGE
                # (last two) for 2x stream bandwidth without putting the
                # first tile behind the weight loads; group 1 stays off the
                # scalar queue (its issue slots would be behind the exps)
                eng = nc.scalar if (gp == 0 and dg >= 2) else nc.gpsimd
                eng.dma_start(
                    xt[:], x2v[:, _dgr * dg:_dgr * (dg + 1), gsl])
                for dd in range(_dgr):
                    d = _dgr * dg + dd
                    for sg in range(2):
                        nc.tensor.matmul(
                            pk[:, sg * 512:(sg + 1) * 512],
                            wk_sb[:, d, :],
                            xt[:, dd, sg * 512:(sg + 1) * 512],
                            start=(d == 0), stop=(d == KD1 - 1))
                        nc.tensor.matmul(
                            pv[:, sg * 512:(sg + 1) * 512],
                            wv_sb[:, d, :],
                            xt[:, dd, sg * 512:(sg + 1) * 512],
                            start=(d == 0), stop=(d == KD1 - 1))
                if dg == KD1 // _dgr - 1:
                    pk, pv = kv_ps.pop(gp)
                    nc.vector.tensor_copy(kTb[:, gsl], pk[:])
                    nc.vector.tensor_copy(vT[:, gsl], pv[:])

            def q_dg(gp, dg):
                gsl = slice(gp * 1024, (gp + 1) * 1024)
                if dg == 0:
                    q_ps[gp] = pspool.tile([P, 1024], F32, tag="ps",
                                           name=f"pq{gp}")
                pq = q_ps[gp]
                xt = x1pool.tile([P, _dgr, 1024], BF16, tag="xt",
                                 name=f"xt1_{gp}_{dg}")
                nc.sync.dma_start(
                    xt[:], x1v[:, _dgr * dg:_dgr * (dg + 1), gsl])
                for dd in range(_dgr):
                    d = _dgr * dg + dd
                    for sg in range(2):
                        nc.tensor.matmul(
                            pq[:, sg * 512:(sg + 1) * 512],
                            wq_sb[:, d, :],
                            xt[:, dd, sg * 512:(sg + 1) * 512],
                            start=(d == 0), stop=(d == KD1 - 1))
                if dg == KD1 // _dgr - 1:
                    pq = q_ps.pop(gp)
                    nc.vector.tensor_copy(qTb[:, gsl], pq[:])

            # ---------------- V natural layout + ones ----------------
            nc.vector.memset(vP[:, :, DV:DV + 1], 1.0)
            nc.vector.memset(vP[:, :, 2 * DV + 1:2 * DV + 2], 1.0)

            def v_transpose(half):
                ptr = pspool.tile([P, 1024], BF16, tag="ps", name=f"ptr{half}")
                for k in range(8):
                    t = 8 * half + k
                    nc.tensor.transpose(
                        ptr[:, k * P:(k + 1) * P],
                        vT[:, t * P:(t + 1) * P],
                        ident[:])
                for k in range(8):
                    t = 8 * half + k
                    nc.vector.tensor_copy(
                        vP[:, t, 0:DV], ptr[:, k * P:k * P + DV])
                    nc.vector.tensor_copy(
                        vP[:, t, DV + 1:2 * DV + 1],
                        ptr[:, k * P + DV:(k + 1) * P])

            # ---------------- attention ----------------
            # scores read kTb/qTb directly at partition offset h*64
            # (no padded per-head copies; K=64 matmul costs the same).
            po_tiles = {}

            def attn(h, ih, t0, t1):
                if (h, ih) not in po_tiles:
                    po_tiles[(h, ih)] = [
                        popool.tile([DV + 1, 512], F32, tag="po",
                                    name=f"po_{h}_{ih}_{gg}")
                        for gg in range(2)]
                po = po_tiles[(h, ih)]
                hsl = slice(h * DK, (h + 1) * DK)
                for t in range(t0, t1):
                    sps = pspool.tile([P, 1024], F32, tag="ps",
                                      name=f"sps_{h}_{ih}_{t}")
                    for sg in range(2):
                        i0 = ih * 1024 + sg * 512
                        nc.tensor.matmul(
                            sps[:, sg * 512:(sg + 1) * 512],
                            kTb[hsl, t * P:(t + 1) * P],
                            qTb[hsl, i0:i0 + 512],
                            start=True, stop=True)
                    ptt = ptpool.tile([P, 1024], BF16, tag="ptt",
                                      name=f"ptt_{h}_{ih}_{t}")
                    nc.scalar.activation(ptt[:], sps[:], ACT_EXP,
                                         scale=float(SCALE))
                    for sg in range(2):
                        nc.tensor.matmul(
                            po[sg][:],
                            vP[:, t, h * (DV + 1):(h + 1) * (DV + 1)],
                            ptt[:, sg * 512:(sg + 1) * 512],
                            start=(t == 0), stop=(t == S2_T - 1))

            def attn_finalize(h, ih):
                # reciprocal of the sums row (fast ~18-bit DVE approx), K=1
                # matmul broadcast over DV partitions, then scale into the
                # bf16 exchange tile
                po = po_tiles.pop((h, ih))
                for gg in range(2):
                    g = ih * 2 + gg
                    gs = slice(g * 512, (g + 1) * 512)
                    oTf = npool.tile([DV, 512], F32, tag="otf",
                                     name=f"otf_{h}_{g}")
                    nc.vector.tensor_copy(oTf[:], po[gg][0:DV, :])
                    sf = npool.tile([DV + 1, 512], F32, tag="sf",
                                    name=f"sf_{h}_{g}")
                    if int(__import__("os").environ.get("RECIP_LNEXP", "1")):
                        # 1/Z = exp(-ln Z) on the ACT engine (idle at phase
                        # boundaries; ~0.5us/op vs 3.3us DVE reciprocal)
                        sfl = npool.tile([DV + 1, 512], F32, tag="sfl",
                                         name=f"sfl_{h}_{g}")
                        nc.scalar.activation(sfl[DV:DV + 1, :],
                                             po[gg][DV:DV + 1, :], ACT_LN)
                        nc.scalar.activation(sf[DV:DV + 1, :],
                                             sfl[DV:DV + 1, :], ACT_EXP,
                                             scale=-1.0)
                    else:
                        nc.vector.reciprocal(
                            sf[DV:DV + 1, :], po[gg][DV:DV + 1, :])
                    sr = npool.tile([DV + 1, 512], F32R, tag="sr",
                                    name=f"sr_{h}_{g}")
                    nc.vector.tensor_copy(sr[DV:DV + 1, :], sf[DV:DV + 1, :])
                    bc = popool.tile([DV, 512], F32, tag="po",
                                     name=f"bc_{h}_{g}")
                    nc.tensor.matmul(
                        bc[:],
                        ones_t[DV:DV + 1, :],
                        sr[DV:DV + 1, :],
                        start=True, stop=True)
                    nc.vector.tensor_mul(oTh[h][:, gs], oTf[:], bc[:])

            def exchange(h):
                # scatter normalized head rows into this head's A2A buffer
                # then launch the exchange immediately (head 0's AllToAll
                # overlaps head 1's attention compute); all in bf16
                nc.sync.dma_start(
                    a2a_in[h].rearrange("(j p) i -> p j i", p=DV),
                    oTh[h][:].rearrange("p (j i) -> p j i", j=NC_CORES))
                if single_core:
                    nc.sync.dma_start(a2a_out[h][:], a2a_in[h][:])
                else:
                    nc.gpsimd.collective_compute(
                        "AllToAll", mybir.AluOpType.bypass,
                        replica_groups=[list(range(NC_CORES))],
                        ins=[a2a_in[h][:].opt()],
                        outs=[a2a_out[h][:].opt()],
                    )

            # out-proj lhsT row blocks: ltsH[h] k-block j packs heads
            # (4j+2h, 4j+2h+2)... i.e. a2a_out[h] blocks (2j, 2j+1) on
            # partition halves; Wo is host-reordered to match.
            ltsH = [[lpool.tile([P, KDV // 2, P], BF16, tag=f"lt{h}_{it}",
                                name=f"lt{h}_{it}")
                     for it in range(SPC // P)] for h in range(HPC)]

            def lt_load(h):
                v = a2a_out[h].rearrange("(k two p) i -> p two k i",
                                         p=DV, two=2)
                for it in range(SPC // P):
                    isl = slice(it * P, (it + 1) * P)
                    nc.sync.dma_start(ltsH[h][it][0:DV, :, :],
                                      v[:, 0, :, isl])
                    nc.sync.dma_start(ltsH[h][it][DV:2 * DV, :, :],
                                      v[:, 1, :, isl])

            # ---------------- output projection + epilogue ----------------
            py_tiles = {}

            def outproj(h):
                # k-blocks [4h, 4h+4) of the reordered Wo; h=0 half starts
                # as soon as the first AllToAll has landed (overlaps the
                # second one), h=1 half finishes the accumulation + bias
                for it in range(SPC // P):
                    if h == 0:
                        py_tiles[it] = pspool.tile([P, D1], F32, tag="ps",
                                                   name=f"py{it}")
                    py = py_tiles[it]
                    lt = ltsH[h][it]
                    for k in range(KDV // 2):
                        for ng in range(2):
                            nc.tensor.matmul(
                                py[:, ng * 512:(ng + 1) * 512],
                                lt[:, k, :],
                                wo_sb[:, 4 * h + k, ng * 512:(ng + 1) * 512],
                                start=(h == 0 and k == 0), stop=False,
                                skip_group_check=True)
                    if h == HPC - 1:
                        # bias folded into the same PSUM accumulation
                        for ng in range(2):
                            nc.tensor.matmul(
                                py[:, ng * 512:(ng + 1) * 512],
                                ones_r[:],
                                bo_r[:, ng * 512:(ng + 1) * 512],
                                start=False, stop=True,
                                skip_group_check=True)

            def epilogue():
                for it in range(SPC // P):
                    py = py_tiles.pop(it)
                    ysb = ypool.tile([P, D1], F32, tag="ysb", bufs=2)
                    yml = ypool.tile([P, D1], F32, tag="yml", bufs=2)
                    nc.vector.tensor_scalar_mul(yml[:], py[:], NEG_SLOPE)
                    nc.vector.tensor_tensor(ysb[:], py[:], yml[:], MAX)
                    nc.sync.dma_start(out[it * P:(it + 1) * P, :], ysb[:])

            # ---------------- emission ----------------
            # group-0 projections with q/kv interleaved (both DMA queues
            # stream in parallel), then the first attention half runs while
            # group-1 x tiles prefetch.
            for dg in range(KD1 // _dgr):
                kv_dg(0, dg)
                q_dg(0, dg)
            v_transpose(0)
            attn(0, 0, 0, 8)
            for dg in range(KD1 // _dgr):
                kv_dg(1, dg)
                q_dg(1, dg)
            v_transpose(1)
            # wo load on the sync queue right after the x1 stream drains
            nc.sync.dma_start(wo_sb[:], woT[:])
            attn(0, 0, 8, S2_T)
            # each finalize is emitted AFTER the first 3 score tiles of the
            # next phase so its ~4.5us Ln/Exp table chain hides under real
            # matmuls instead of stalling the tensor queue
            attn(0, 1, 0, 3)
            attn_finalize(0, 0)
            attn(0, 1, 3, S2_T)
            attn(1, 0, 0, 3)
            attn_finalize(0, 1)
            exchange(0)
            lt_load(0)
            attn(1, 0, 3, S2_T)
            attn(1, 1, 0, 3)
            attn_finalize(1, 0)
            attn(1, 1, 3, S2_T)
            # even-half out-proj before the last finalize + second exchange:
            # it has no dependence on them and fills the tensor queue while
            # the finalize chain and then the AllToAll run
            outproj(0)
            attn_finalize(1, 1)
            exchange(1)
            lt_load(1)
            outproj(1)
            epilogue()

    nc.compile()
    return nc


_NC_CACHE = {}


def _get_nc():
    if "nc" not in _NC_CACHE:
        _NC_CACHE["nc"] = build()
    return _NC_CACHE["nc"]


# out-proj head order after the per-head AllToAll: lt k-blocks pack heads
# (0,2),(4,6),(8,10),(12,14) then (1,3),(5,7),(9,11),(13,15)
WO_HEAD_ORDER = [0, 2, 4, 6, 8, 10, 12, 14, 1, 3, 5, 7, 9, 11, 13, 15]


def _shuffle_w(wT):
    """[K, M] weight (contraction-major) -> [128, K//128, M] partition-major
    so the SBUF load is one big descriptor per partition."""
    K, M = wT.shape
    return np.ascontiguousarray(
        wT.reshape(K // P, P, M).transpose(1, 0, 2))


def make_in_maps(x1, x2, Wq, Wk, Wv, Wo, bo):
    import ml_dtypes
    cast = lambda a: a.astype(ml_dtypes.bfloat16)  # noqa: E731
    x1 = np.asarray(x1, dtype=np.float32)
    x2 = np.asarray(x2, dtype=np.float32)
    Wq = np.asarray(Wq, dtype=np.float32)
    Wk = np.asarray(Wk, dtype=np.float32)
    Wv = np.asarray(Wv, dtype=np.float32)
    Wo = np.asarray(Wo, dtype=np.float32)
    bo = np.asarray(bo, dtype=np.float32)
    x1T = cast(np.ascontiguousarray(x1.T))
    x2T = cast(np.ascontiguousarray(x2.T))
    woT_full = np.ascontiguousarray(Wo.T)  # [H*DV, D1]
    woT = cast(_shuffle_w(np.concatenate(
        [woT_full[h * DV:(h + 1) * DV] for h in WO_HEAD_ORDER], axis=0)))
    bo_bc = np.ascontiguousarray(bo.reshape(1, D1))
    in_maps = []
    for c in range(NC_CORES):
        sl = slice(EPC * c, EPC * (c + 1))
        in_maps.append({
            "x1T": x1T,
            "x2T": x2T,
            "wqT": cast(_shuffle_w(np.ascontiguousarray(Wq[sl, :].T))),
            "wkT": cast(_shuffle_w(np.ascontiguousarray(Wk[sl, :].T))),
            "wvT": cast(_shuffle_w(np.ascontiguousarray(Wv[sl, :].T))),
            "woT": woT,
            "bo_bc": bo_bc,
        })
    return in_maps


def _install_profile_shim():
    """The image's antenv lacks axon_hooks; shim it so trace=True can pull
    NTFF profiles (exec_time_ns) through the axon tunnel."""
    import sys as _sys
    import types as _types
    try:
        from antenv.axon_hooks import get_axon_ntff_profile_hook  # noqa: F401
        return
    except ImportError:
        pass
    try:
        from trn_agent_boot.trn_boot import _ntff_profile_via_ctypes
        hook = _ntff_profile_via_ctypes("/opt/axon/libaxon_pjrt.so")
        mod = _types.ModuleType("antenv.axon_hooks")
        mod.get_axon_ntff_profile_hook = lambda: hook
        mod.set_axon_ntff_profile_hook = lambda h: None
        _sys.modules["antenv.axon_hooks"] = mod
        bass_utils.upload_artifacts = lambda tmpdir: tmpdir
    except Exception:
        pass


def run(inputs, trace=False):
    if trace:
        _install_profile_shim()
    nc = _get_nc()
    in_maps = make_in_maps(**inputs)
    res = bass_utils.run_bass_kernel_spmd(
        nc, in_maps, core_ids=list(range(NC_CORES)), trace=trace)
    full = np.concatenate(
        [res.results[c]["out"] for c in range(NC_CORES)], axis=0)
    return full, res


def kernel(**inputs):
    full, _ = run(inputs, trace=False)
    return full
